# revision 1
# baseline (speedup 1.0000x reference)
"""Corr1d kernel for Trainium2 (Bass/Tile), self-contained.

Math: for x, y of shape [B=8, C=128, H=96, W=320] (fp32), MAX_DISP=10,
the reference computes, per (b, h, w):

    out = 1/(21*128) * sum_c [ x[c,w] * S_y[c,w] + y[c,w] * S_x[c,w] ]
      S_y[c,w] = sum_{d=1..10} y[c,w-d]   (zero padded)
      S_x[c,w] = sum_{d=0..10} x[c,w+d]   (zero padded)

Mapping: data-parallel over B across 8 cores (1 batch per core). C=128 on
SBUF partitions. Rows of W are laid out with 16-element zero pads on each
side (row stride 352) so shifted reads never cross row boundaries. The
sliding-window sums are computed with a single tensor_tensor_scan per
tensor:  state = (v[t] + state) - v[t - win]  which keeps the running
state equal to the causal window sum (bounded, no cancellation). Products
run on DVE in fp16 (2x mode), the channel reduction is a ones-matmul on
the PE accumulating both products into PSUM, and ACT applies the 1/2688
scale on the PSUM->SBUF copy.
"""

import numpy as np

import concourse.bacc as bacc
import concourse.bass as bass  # noqa: F401  (AP types re-exported)
import concourse.mybir as mybir
from concourse.tile import TileContext
from concourse.bass_utils import run_bass_kernel_spmd

B, C, H, W = 8, 128, 96, 320
MAX_DISP = 10
N_CORES = 8

HC = 16                 # H rows per chunk
NCHUNK = H // HC        # 6
PAD = 16                # per-row left/right zero pad
RS = PAD + W + PAD      # 352 row stride in SBUF
G = 16                  # leading guard columns (zeros)
NP = HC * RS            # 5632 scanned columns per chunk (11 * 512)
F = G + NP              # X/Y buffer width
FA = F + 16             # A/B buffer width (py reads A at +10)
SCALE = 1.0 / ((2 * MAX_DISP + 1) * C)

# Compute dtype on chip: float16 (DVE 2x products, half DMA traffic).
# Flip to float32 for an exactness A/B check (slower).
COMPUTE_DT = mybir.dt.float16

_CACHE = {}


def _build(compute_dt):
    fdt = compute_dt
    # Bacc (not plain Bass): its finalize() runs generate_event_semaphores,
    # which splits multi-wait sync conditions into CTRL_ES instructions —
    # DVE instruction formats only have one embedded wait slot.
    nc = bacc.Bacc("TRN2", target_bir_lowering=False)
    x = nc.dram_tensor("x", [C, H, W], fdt, kind="ExternalInput")
    y = nc.dram_tensor("y", [C, H, W], fdt, kind="ExternalInput")
    out = nc.dram_tensor("out", [H, W], mybir.dt.float32, kind="ExternalOutput")

    ADD = mybir.AluOpType.add
    SUB = mybir.AluOpType.subtract
    MUL = mybir.AluOpType.mult

    with TileContext(nc) as tc:
        with (
            tc.tile_pool(name="sb", bufs=1) as sb,
            tc.tile_pool(name="ps", bufs=6, space="PSUM") as ps,
            tc.tile_pool(name="ob", bufs=2) as ob,
        ):
            # Persistent buffers; pads stay zero after the initial memset
            # because DMA only ever writes the row-data column ranges.
            Xb = [sb.tile([128, F], fdt, name=f"xbuf{i}") for i in range(2)]
            Yb = [sb.tile([128, F], fdt, name=f"ybuf{i}") for i in range(2)]
            Aw = sb.tile([128, FA], fdt, name="awin")   # causal win-11 of x
            Bw = sb.tile([128, FA], fdt, name="bwin")   # causal win-10 of y (shifted +1)
            px = sb.tile([128, NP], fdt, name="px")
            py = sb.tile([128, NP], fdt, name="py")
            ones = sb.tile([128, 1], fdt, name="ones")

            nc.vector.memset(ones[:], 1.0)
            for t in (*Xb, *Yb, Aw, Bw):
                nc.vector.memset(t[:], 0.0)

            for i in range(NCHUNK):
                X = Xb[i % 2]
                Y = Yb[i % 2]
                h0 = i * HC

                # Load x/y rows into padded slots: row r data at
                # columns [G + r*RS + PAD, G + r*RS + PAD + W).
                dstx = X[:, G:F].rearrange("p (h w) -> p h w", h=HC)[
                    :, :, PAD : PAD + W
                ]
                nc.sync.dma_start(dstx, x[:, h0 : h0 + HC, :])
                dsty = Y[:, G:F].rearrange("p (h w) -> p h w", h=HC)[
                    :, :, PAD : PAD + W
                ]
                nc.sync.dma_start(dsty, y[:, h0 : h0 + HC, :])

                # Fences: the scan ISA struct (S2S2D2_STT, 64B) has no room
                # for multiple embedded sync waits, so hand the cross-engine
                # DMA waits to plain TT ops. Each fence reads a DMA-written
                # column and writes an (always zero) guard column that the
                # scan's data1 AP covers — a true RAW dep on the same engine,
                # so the scans need no waits of their own. Column 0 is a
                # permanent zero, so data * 0 keeps the guard at zero.
                nc.vector.tensor_tensor(
                    X[:, 5:6], X[:, G + PAD : G + PAD + 1], X[:, 0:1], MUL
                )
                nc.vector.tensor_tensor(
                    Y[:, 6:7], Y[:, G + PAD : G + PAD + 1], Y[:, 0:1], MUL
                )

                # A[t] = sum_{v=t-10..t} X[v]  via state=(X[t]+state)-X[t-11]
                nc.vector.tensor_tensor_scan(
                    Aw[:, G:F], X[:, G:F], X[:, G - 11 : F - 11], 0.0, ADD, SUB
                )
                # B[t+1] = sum_{v=t-9..t} Y[v] via state=(Y[t]+state)-Y[t-10]
                # (output shifted +1 so S_y[q] lands at column q, keeping the
                #  product reads 4B-aligned for the DVE 2x mode)
                nc.vector.tensor_tensor_scan(
                    Bw[:, G + 1 : F + 1], Y[:, G:F], Y[:, G - 10 : F - 10], 0.0, ADD, SUB
                )

                # px[q] = x[q] * S_y[q],  py[q] = y[q] * S_x[q] = y[q]*A[q+10]
                nc.vector.tensor_tensor(px[:, :], X[:, G:F], Bw[:, G:F], MUL)
                nc.vector.tensor_tensor(py[:, :], Y[:, G:F], Aw[:, G + 10 : F + 10], MUL)

                # Channel reduction: psum[0, q] = sum_c (px + py), then scale.
                outsb = ob.tile([1, NP], mybir.dt.float32, tag="outsb")
                for s in range(NP // 512):
                    pt = ps.tile([1, 512], mybir.dt.float32, tag="ps")
                    sl = slice(512 * s, 512 * (s + 1))
                    nc.tensor.matmul(pt[:], ones[:], px[:, sl], start=True, stop=False)
                    nc.tensor.matmul(pt[:], ones[:], py[:, sl], start=False, stop=True)
                    nc.scalar.mul(outsb[:, sl], pt[:], SCALE)

                # Extract the valid W columns of each row.
                src = outsb[:, :].rearrange("p (h w) -> p h w", h=HC)[
                    :, :, PAD : PAD + W
                ]
                nc.sync.dma_start(out[h0 : h0 + HC, :], src)

    # Bacc.finalize() runs the compile pipeline (register allocation,
    # generate_event_semaphores wait-splitting). The axon/PJRT run path
    # takes a prebuilt module and never calls it, so do it here.
    nc.finalize()
    return nc


def _get_nc():
    key = ("nc", str(COMPUTE_DT))
    if key not in _CACHE:
        _CACHE[key] = _build(COMPUTE_DT)
    return _CACHE[key]


def _np_dt():
    return np.float16 if COMPUTE_DT == mybir.dt.float16 else np.float32


def run(inputs, trace=False, trace_cores=None):
    """Run on hardware; returns (out [B,H,W] fp32, BassKernelResults)."""
    x = np.asarray(inputs["x"], dtype=np.float32)
    y = np.asarray(inputs["y"], dtype=np.float32)
    assert x.shape == (B, C, H, W) and y.shape == (B, C, H, W)
    dt = _np_dt()
    xh = np.ascontiguousarray(x.astype(dt))
    yh = np.ascontiguousarray(y.astype(dt))
    in_maps = [{"x": xh[b], "y": yh[b]} for b in range(B)]
    nc = _get_nc()
    res = run_bass_kernel_spmd(
        nc,
        in_maps,
        core_ids=list(range(N_CORES)),
        trace=trace,
        trace_cores=trace_cores,
    )
    outs = np.stack([r["out"] for r in res.results], axis=0).astype(np.float32)
    return outs, res


def kernel(**inputs) -> np.ndarray:
    out, _ = run(inputs, trace=False)
    return out



# revision 9
# speedup vs baseline: 1.2030x; 1.2030x over previous
"""Corr1d kernel for Trainium2 (Bass/Tile), self-contained.

Math: for x, y of shape [B=8, C=128, H=96, W=320] (fp32), MAX_DISP=10,
the reference computes, per (b, h, w):

    out = 1/(21*128) * sum_c [ x[c,w] * S_y[c,w] + y[c,w] * S_x[c,w] ]
      S_y[c,w] = sum_{d=1..10} y[c,w-d]   (zero padded)
      S_x[c,w] = sum_{d=0..10} x[c,w+d]   (zero padded)

Mapping: data-parallel over B across 8 cores (1 batch per core). C=128 on
SBUF partitions. Rows of W are laid out with 16-element zero pads on each
side (row stride 352) so shifted reads never cross row boundaries. The
sliding-window sums are computed with a single tensor_tensor_scan per
tensor:  state = (v[t] + state) - v[t - win]  which keeps the running
state equal to the causal window sum (bounded, no cancellation). Products
run on DVE in fp16 (2x mode), the channel reduction is a ones-matmul on
the PE accumulating both products into PSUM, and ACT applies the 1/2688
scale on the PSUM->SBUF copy.
"""

import numpy as np

import concourse.bacc as bacc
import concourse.bass as bass  # noqa: F401  (AP types re-exported)
import concourse.mybir as mybir
from concourse.tile import TileContext
from concourse.bass_utils import run_bass_kernel_spmd

B, C, H, W = 8, 128, 96, 320
MAX_DISP = 10
N_CORES = 8

HC = 16                 # H rows per chunk
NCHUNK = H // HC        # 6
PAD = 12                # per-row leading zero gap (>= 11 for A-scan lookback)
RS = PAD + W            # 332 row stride in SBUF (row data at [PAD, RS))
G = 16                  # leading guard columns (zeros)
NP = HC * RS            # 5312 product columns per chunk
F = G + NP              # X/Y data end
FA = F + 16             # A/B/X/Y buffer width (A-scan extends +10 past F)
SCALE = 1.0 / ((2 * MAX_DISP + 1) * C)

# Compute dtype on chip: float16 (DVE 2x products, half DMA traffic).
# Flip to float32 for an exactness A/B check (slower).
COMPUTE_DT = mybir.dt.float16

_CACHE = {}


def _build(compute_dt):
    fdt = compute_dt
    # Bacc (not plain Bass): its finalize() runs generate_event_semaphores,
    # which splits multi-wait sync conditions into CTRL_ES instructions —
    # DVE instruction formats only have one embedded wait slot.
    nc = bacc.Bacc("TRN2", target_bir_lowering=False)
    x = nc.dram_tensor("x", [C, H, W], fdt, kind="ExternalInput")
    y = nc.dram_tensor("y", [C, H, W], fdt, kind="ExternalInput")
    out = nc.dram_tensor("out", [H, W], mybir.dt.float32, kind="ExternalOutput")

    ADD = mybir.AluOpType.add
    SUB = mybir.AluOpType.subtract
    MUL = mybir.AluOpType.mult

    with TileContext(nc) as tc:
        with (
            tc.tile_pool(name="sb", bufs=1) as sb,
            tc.tile_pool(name="ps", bufs=5, space="PSUM") as ps,
            tc.tile_pool(name="psr", bufs=2, space="PSUM") as psr,
            tc.tile_pool(name="ob", bufs=2) as ob,
        ):
            # Persistent buffers; pads stay zero after the initial memset
            # because DMA only ever writes the row-data column ranges.
            Xb = [sb.tile([128, FA], fdt, name=f"xbuf{i}") for i in range(2)]
            Yb = [sb.tile([128, FA], fdt, name=f"ybuf{i}") for i in range(2)]
            Aw = sb.tile([128, FA], fdt, name="awin")   # causal win-11 of x
            Bw = sb.tile([128, FA], fdt, name="bwin")   # causal win-10 of y (shifted +1)
            px = sb.tile([128, NP], fdt, name="px")
            py = sb.tile([128, NP], fdt, name="py")
            ones = sb.tile([128, 1], fdt, name="ones")

            nc.vector.memset(ones[:], 1.0)
            # Only the pad columns need zeroing: head guard [0, G+PAD), each
            # row's leading gap, and the tail [F, FA). Data columns are
            # DMA-written before any read. GPSIMD does it to keep DVE free.
            for t in (*Xb, *Yb):
                nc.gpsimd.memset(t[:, 0 : G + PAD], 0.0)
                gaps = t[:, G:F].rearrange("p (h w) -> p h w", h=HC)[:, :, 0:PAD]
                nc.gpsimd.memset(gaps, 0.0)
                nc.gpsimd.memset(t[:, F:FA], 0.0)
            # Bw[G] is read by px but never written by the B-scan; Aw's
            # guard/tail cells that py reads are all scan-written or
            # multiplied by structural zeros, but zero them anyway to keep
            # NaN garbage out.
            nc.gpsimd.memset(Bw[:, G : G + 1], 0.0)
            nc.gpsimd.memset(Aw[:, 0 : G + 10], 0.0)
            nc.gpsimd.memset(Aw[:, F + 10 : FA], 0.0)

            for i in range(NCHUNK):
                X = Xb[i % 2]
                Y = Yb[i % 2]
                h0 = i * HC

                # Load x/y rows into padded slots: row r data at
                # columns [G + r*RS + PAD, G + (r+1)*RS).
                dstx = X[:, G:F].rearrange("p (h w) -> p h w", h=HC)[
                    :, :, PAD : RS
                ]
                nc.sync.dma_start(dstx, x[:, h0 : h0 + HC, :])
                dsty = Y[:, G:F].rearrange("p (h w) -> p h w", h=HC)[
                    :, :, PAD : RS
                ]
                nc.sync.dma_start(dsty, y[:, h0 : h0 + HC, :])

                # Fences: the scan ISA struct (S2S2D2_STT, 64B) has no room
                # for multiple embedded sync waits, so hand the cross-engine
                # DMA waits to plain TT ops. Each fence reads a DMA-written
                # column and writes an (always zero) guard column that the
                # scan's data1 AP covers — a true RAW dep on the same engine,
                # so the scans need no waits of their own. Column 0 is a
                # permanent zero, so data * 0 keeps the guard at zero.
                nc.vector.tensor_tensor(
                    X[:, 5:6], X[:, G + PAD : G + PAD + 1], X[:, 0:1], MUL
                )
                nc.vector.tensor_tensor(
                    Y[:, 6:7], Y[:, G + PAD : G + PAD + 1], Y[:, 0:1], MUL
                )

                # A[t] = sum_{v=t-10..t} X[v]  via state=(X[t]+state)-X[t-11]
                # Extended +10 past F so py's read A[q+10] is scan-written for
                # the last row's tail (X[F:F+10) is zero tail).
                nc.vector.tensor_tensor_scan(
                    Aw[:, G : F + 10],
                    X[:, G : F + 10],
                    X[:, G - 11 : F - 1],
                    0.0,
                    ADD,
                    SUB,
                )
                # B[t+1] = sum_{v=t-9..t} Y[v] via state=(Y[t]+state)-Y[t-10]
                # (output shifted +1 so S_y[q] lands at column q, keeping the
                #  product reads 4B-aligned for the DVE 2x mode)
                nc.vector.tensor_tensor_scan(
                    Bw[:, G + 1 : F + 1], Y[:, G:F], Y[:, G - 10 : F - 10], 0.0, ADD, SUB
                )

                # px[q] = x[q] * S_y[q],  py[q] = y[q] * S_x[q] = y[q]*A[q+10]
                nc.vector.tensor_tensor(px[:, :], X[:, G:F], Bw[:, G:F], MUL)
                nc.vector.tensor_tensor(py[:, :], Y[:, G:F], Aw[:, G + 10 : F + 10], MUL)

                # Channel reduction: psum[0, q] = sum_c (px + py), then scale.
                outsb = ob.tile([1, NP], mybir.dt.float32, tag="outsb")
                for s in range((NP + 511) // 512):
                    lo = 512 * s
                    hi = min(lo + 512, NP)
                    pool = ps if hi - lo == 512 else psr
                    pt = pool.tile([1, hi - lo], mybir.dt.float32, tag="pst")
                    sl = slice(lo, hi)
                    nc.tensor.matmul(pt[:], ones[:], px[:, sl], start=True, stop=False)
                    nc.tensor.matmul(pt[:], ones[:], py[:, sl], start=False, stop=True)
                    nc.scalar.mul(outsb[:, sl], pt[:], SCALE)

                # Extract the valid W columns of each row.
                src = outsb[:, :].rearrange("p (h w) -> p h w", h=HC)[
                    :, :, PAD:RS
                ]
                nc.sync.dma_start(out[h0 : h0 + HC, :], src)

    # Bacc.finalize() runs the compile pipeline (register allocation,
    # generate_event_semaphores wait-splitting). The axon/PJRT run path
    # takes a prebuilt module and never calls it, so do it here.
    nc.finalize()
    return nc


def _get_nc():
    key = ("nc", str(COMPUTE_DT))
    if key not in _CACHE:
        _CACHE[key] = _build(COMPUTE_DT)
    return _CACHE[key]


def _np_dt():
    return np.float16 if COMPUTE_DT == mybir.dt.float16 else np.float32


def run(inputs, trace=False, trace_cores=None):
    """Run on hardware; returns (out [B,H,W] fp32, BassKernelResults)."""
    x = np.asarray(inputs["x"], dtype=np.float32)
    y = np.asarray(inputs["y"], dtype=np.float32)
    assert x.shape == (B, C, H, W) and y.shape == (B, C, H, W)
    dt = _np_dt()
    xh = np.ascontiguousarray(x.astype(dt))
    yh = np.ascontiguousarray(y.astype(dt))
    in_maps = [{"x": xh[b], "y": yh[b]} for b in range(B)]
    nc = _get_nc()
    res = run_bass_kernel_spmd(
        nc,
        in_maps,
        core_ids=list(range(N_CORES)),
        trace=trace,
        trace_cores=trace_cores,
    )
    outs = np.stack([r["out"] for r in res.results], axis=0).astype(np.float32)
    return outs, res


def kernel(**inputs) -> np.ndarray:
    out, _ = run(inputs, trace=False)
    return out



# revision 11
# speedup vs baseline: 1.2059x; 1.0024x over previous
"""Corr1d kernel for Trainium2 (Bass/Tile), self-contained.

Math: for x, y of shape [B=8, C=128, H=96, W=320] (fp32), MAX_DISP=10,
the reference computes, per (b, h, w):

    out = 1/(21*128) * sum_c [ x[c,w] * S_y[c,w] + y[c,w] * S_x[c,w] ]
      S_y[c,w] = sum_{d=1..10} y[c,w-d]   (zero padded)
      S_x[c,w] = sum_{d=0..10} x[c,w+d]   (zero padded)

Mapping: data-parallel over B across 8 cores (1 batch per core). C=128 on
SBUF partitions. Rows of W are laid out with 16-element zero pads on each
side (row stride 352) so shifted reads never cross row boundaries. The
sliding-window sums are computed with a single tensor_tensor_scan per
tensor:  state = (v[t] + state) - v[t - win]  which keeps the running
state equal to the causal window sum (bounded, no cancellation). Products
run on DVE in fp16 (2x mode), the channel reduction is a ones-matmul on
the PE accumulating both products into PSUM, and ACT applies the 1/2688
scale on the PSUM->SBUF copy.
"""

import numpy as np

import concourse.bacc as bacc
import concourse.bass as bass  # noqa: F401  (AP types re-exported)
import concourse.mybir as mybir
from concourse.tile import TileContext
from concourse.bass_utils import run_bass_kernel_spmd

B, C, H, W = 8, 128, 96, 320
MAX_DISP = 10
N_CORES = 8

HC = 16                 # H rows per chunk
NCHUNK = H // HC        # 6
PAD = 12                # per-row leading zero gap (>= 11 for A-scan lookback)
RS = PAD + W            # 332 row stride in SBUF (row data at [PAD, RS))
G = 16                  # leading guard columns (zeros)
NP = HC * RS            # 5312 product columns per chunk
F = G + NP              # X/Y data end
FA = F + 16             # A/B/X/Y buffer width (A-scan extends +10 past F)
SCALE = 1.0 / ((2 * MAX_DISP + 1) * C)

# Compute dtype on chip: float16 (DVE 2x products, half DMA traffic).
# Flip to float32 for an exactness A/B check (slower).
COMPUTE_DT = mybir.dt.float16

_CACHE = {}


def _build(compute_dt):
    fdt = compute_dt
    # Bacc (not plain Bass): its finalize() runs generate_event_semaphores,
    # which splits multi-wait sync conditions into CTRL_ES instructions —
    # DVE instruction formats only have one embedded wait slot.
    nc = bacc.Bacc("TRN2", target_bir_lowering=False)
    x = nc.dram_tensor("x", [C, H, W], fdt, kind="ExternalInput")
    y = nc.dram_tensor("y", [C, H, W], fdt, kind="ExternalInput")
    out = nc.dram_tensor("out", [H, W], mybir.dt.float32, kind="ExternalOutput")

    ADD = mybir.AluOpType.add
    SUB = mybir.AluOpType.subtract
    MUL = mybir.AluOpType.mult

    with TileContext(nc) as tc:
        with (
            tc.tile_pool(name="sb", bufs=1) as sb,
            tc.tile_pool(name="ps", bufs=5, space="PSUM") as ps,
            tc.tile_pool(name="psr", bufs=2, space="PSUM") as psr,
            tc.tile_pool(name="ob", bufs=2) as ob,
        ):
            # Persistent buffers; pads stay zero after the initial memset
            # because DMA only ever writes the row-data column ranges.
            Xb = [sb.tile([128, FA], fdt, name=f"xbuf{i}") for i in range(2)]
            Yb = [sb.tile([128, FA], fdt, name=f"ybuf{i}") for i in range(2)]
            Aw = sb.tile([128, FA], fdt, name="awin")   # causal win-11 of x
            Bw = sb.tile([128, FA], fdt, name="bwin")   # causal win-10 of y (shifted +1)
            px = sb.tile([128, NP], fdt, name="px")
            py = sb.tile([128, NP], fdt, name="py")
            ones = sb.tile([128, 1], fdt, name="ones")

            nc.vector.memset(ones[:], 1.0)
            # Only the pad columns need zeroing: head guard [0, G+PAD), each
            # row's leading gap, and the tail [F, FA). Data columns are
            # DMA-written before any read. GPSIMD does it to keep DVE free.
            for t in (*Xb, *Yb):
                nc.gpsimd.memset(t[:, 0 : G + PAD], 0.0)
                gaps = t[:, G:F].rearrange("p (h w) -> p h w", h=HC)[:, :, 0:PAD]
                nc.gpsimd.memset(gaps, 0.0)
                nc.gpsimd.memset(t[:, F:FA], 0.0)
            # Bw[G] is read by px but never written by the B-scan; Aw's
            # guard/tail cells that py reads are all scan-written or
            # multiplied by structural zeros, but zero them anyway to keep
            # NaN garbage out.
            nc.gpsimd.memset(Bw[:, G : G + 1], 0.0)
            nc.gpsimd.memset(Aw[:, 0 : G + 10], 0.0)
            nc.gpsimd.memset(Aw[:, F + 10 : FA], 0.0)

            for i in range(NCHUNK):
                X = Xb[i % 2]
                Y = Yb[i % 2]
                h0 = i * HC

                # Load x/y rows into padded slots: row r data at
                # columns [G + r*RS + PAD, G + (r+1)*RS).
                dstx = X[:, G:F].rearrange("p (h w) -> p h w", h=HC)[
                    :, :, PAD : RS
                ]
                nc.sync.dma_start(dstx, x[:, h0 : h0 + HC, :])
                dsty = Y[:, G:F].rearrange("p (h w) -> p h w", h=HC)[
                    :, :, PAD : RS
                ]
                nc.sync.dma_start(dsty, y[:, h0 : h0 + HC, :])

                # Fences: the scan ISA struct (S2S2D2_STT, 64B) has no room
                # for multiple embedded sync waits, so hand the cross-engine
                # DMA waits to plain TT ops. Each fence reads a DMA-written
                # column and writes an (always zero) guard column that the
                # scan's data1 AP covers — a true RAW dep on the same engine,
                # so the scans need no waits of their own. Column 0 is a
                # permanent zero, so data * 0 keeps the guard at zero.
                nc.vector.tensor_tensor(
                    X[:, 5:6], X[:, G + PAD : G + PAD + 1], X[:, 0:1], MUL
                )
                nc.vector.tensor_tensor(
                    Y[:, 6:7], Y[:, G + PAD : G + PAD + 1], Y[:, 0:1], MUL
                )

                # A[t] = sum_{v=t-10..t} X[v]  via state=(X[t]+state)-X[t-11]
                # Extended +10 past F so py's read A[q+10] is scan-written for
                # the last row's tail (X[F:F+10) is zero tail).
                nc.vector.tensor_tensor_scan(
                    Aw[:, G : F + 10],
                    X[:, G : F + 10],
                    X[:, G - 11 : F - 1],
                    0.0,
                    ADD,
                    SUB,
                )
                # B[t+1] = sum_{v=t-9..t} Y[v] via state=(Y[t]+state)-Y[t-10]
                # (output shifted +1 so S_y[q] lands at column q, keeping the
                #  product reads 4B-aligned for the DVE 2x mode)
                nc.vector.tensor_tensor_scan(
                    Bw[:, G + 1 : F + 1], Y[:, G:F], Y[:, G - 10 : F - 10], 0.0, ADD, SUB
                )

                # px[q] = x[q] * S_y[q],  py[q] = y[q] * S_x[q] = y[q]*A[q+10]
                nc.vector.tensor_tensor(px[:, :], X[:, G:F], Bw[:, G:F], MUL)
                nc.vector.tensor_tensor(py[:, :], Y[:, G:F], Aw[:, G + 10 : F + 10], MUL)

                # Channel reduction: psum[0, q] = sum_c (px + py), then scale.
                outsb = ob.tile([1, NP], mybir.dt.float32, tag="outsb")
                for s in range((NP + 511) // 512):
                    lo = 512 * s
                    hi = min(lo + 512, NP)
                    pool = ps if hi - lo == 512 else psr
                    pt = pool.tile([1, hi - lo], mybir.dt.float32, tag="pst")
                    sl = slice(lo, hi)
                    nc.tensor.matmul(pt[:], ones[:], px[:, sl], start=True, stop=False)
                    nc.tensor.matmul(pt[:], ones[:], py[:, sl], start=False, stop=True)
                    nc.scalar.mul(outsb[:, sl], pt[:], SCALE)

                # Extract the valid W columns of each row.
                src = outsb[:, :].rearrange("p (h w) -> p h w", h=HC)[
                    :, :, PAD:RS
                ]
                nc.sync.dma_start(out[h0 : h0 + HC, :], src)

    # Bacc.finalize() runs the compile pipeline (register allocation,
    # generate_event_semaphores wait-splitting). The axon/PJRT run path
    # takes a prebuilt module and never calls it, so do it here.
    nc.finalize()
    return nc


def _get_nc():
    key = ("nc", str(COMPUTE_DT))
    if key not in _CACHE:
        _CACHE[key] = _build(COMPUTE_DT)
    return _CACHE[key]


def _np_dt():
    return np.float16 if COMPUTE_DT == mybir.dt.float16 else np.float32


def run(inputs, trace=False, trace_cores=None):
    """Run on hardware; returns (out [B,H,W] fp32, BassKernelResults)."""
    x = np.asarray(inputs["x"], dtype=np.float32)
    y = np.asarray(inputs["y"], dtype=np.float32)
    assert x.shape == (B, C, H, W) and y.shape == (B, C, H, W)
    dt = _np_dt()
    xh = np.ascontiguousarray(x.astype(dt))
    yh = np.ascontiguousarray(y.astype(dt))
    in_maps = [{"x": xh[b], "y": yh[b]} for b in range(B)]
    nc = _get_nc()
    res = run_bass_kernel_spmd(
        nc,
        in_maps,
        core_ids=list(range(N_CORES)),
        trace=trace,
        trace_cores=trace_cores,
    )
    outs = np.stack([r["out"] for r in res.results], axis=0).astype(np.float32)
    return outs, res


def kernel(**inputs) -> np.ndarray:
    out, _ = run(inputs, trace=False)
    return out

